# revision 1
# baseline (speedup 1.0000x reference)
"""Trainium2 Bass kernel for nn_EquivariantCorrectionHead.

Math (reference): two chained e3nn-style fully-connected tensor products on
irreps 128x0e + 8x2e -> Hx0e + Hx2e -> 1x2e, batch B=2048.

Strategy: data-parallel over batch across 8 NeuronCores (weights replicated).
Per core (256 batch rows = 2 tiles of 128):
  - y0 (Hx0e): bilinear form s W000 s done as on-chip pair products
    S2p[z,(u,v)] (upper block-triangle, 16-granularity, host-packed weights
    fold the transpose) -> PE transpose chunks -> accumulated f32r matmuls.
    The 8x2e invariant contraction (T2 @ W220) accumulates into the same PSUM.
  - y2 (Hx2e): W022/W202 paths fused on host into one weight; M = s @ W_PR
    matmul, then per-sample contraction with t on DVE. W222 path via pair
    products + sparse C222 accumulation -> small matmuls.
  - tp2: V022/V202 fused on host; per-sample contractions on DVE; V222 via
    D = y2 V222 matmuls and F products; final C222 combination as a matmul.
All matmuls run in float32r (full PE rate at N>=256); storage is fp32.
"""

import os
import sys
import numpy as np

sys.path.insert(0, "/opt/trn_rl_repo")

B = 2048
N_S = 128
H = 256
N_CORES = 8
ZC = B // N_CORES          # batch rows per core (256)
GR = 16                    # block-triangle granularity
GROUPS = [(GR * g, N_S - GR * g) for g in range(N_S // GR)]
NPACK = sum(GR * w for (_, w) in GROUPS)      # 9216
NCHUNK = NPACK // 128                          # 72

# e3nn path normalization constants (must match reference.py)
N_L2 = 8
C0 = float(np.sqrt(1.0 / (N_S**2 + N_L2**2)))
C2 = float(np.sqrt(5.0 / (2 * N_S * N_L2 + N_L2**2)))
C2B = float(np.sqrt(5.0 / (3 * H**2)))
INV_S5 = float(1.0 / np.sqrt(5.0))


def _w3j_222():
    Q = np.zeros((5, 3, 3))
    s = 1.0 / np.sqrt(2.0)
    Q[0, 0, 1] = Q[0, 1, 0] = s
    Q[1, 1, 2] = Q[1, 2, 1] = s
    Q[2] = np.diag([-1.0, -1.0, 2.0]) / np.sqrt(6.0)
    Q[3, 0, 2] = Q[3, 2, 0] = s
    Q[4, 0, 0] = s
    Q[4, 1, 1] = -s
    C = np.einsum('aij,bjk,cki->abc', Q, Q, Q)
    return (C / np.linalg.norm(C)).astype(np.float32)


C222 = _w3j_222()  # [i, j, k]


def host_prep(scalars, kernel_t2s, W000, W220, W022, W202, W222, V022, V202, V222):
    """Numpy-side packing: layout/constant folding only (plus the trivial
    t-channel sum). Returns dict of DRAM arrays."""
    f = np.float32
    t = np.concatenate([kernel_t2s[:, :7, :], kernel_t2s.sum(1, keepdims=True)], 1)
    t = np.ascontiguousarray(t.reshape(B, 40), dtype=f)

    # Block-triangle-packed W000 (16-granularity). Group g covers
    # u in [16g,16g+16), v in [16g,128). Off-diagonal-block columns fold the
    # transposed weight; diagonal block keeps original entries (both orders
    # of the pair products are present in S2p there).
    packs = []
    for (u0, wv) in GROUPS:
        blk = W000[u0:u0 + GR, u0:, :].copy()              # [16, wv, 256]
        swp = W000[u0:, u0:u0 + GR, :].transpose(1, 0, 2)  # [16, wv, 256]
        blk[:, GR:, :] += swp[:, GR:, :]
        packs.append(blk.reshape(GR * wv, H))
    W000p = np.ascontiguousarray(C0 * np.concatenate(packs, 0), dtype=f)  # [9216,256]

    W_PR = (C2 * INV_S5) * (W022 + W202.transpose(1, 0, 2))  # [128sc, 8l2, 256]
    W_PR = np.ascontiguousarray(W_PR.reshape(N_S * 8, H), dtype=f)
    W220p = np.ascontiguousarray((C0 * INV_S5) * W220.reshape(64, H), dtype=f)
    W222p = np.ascontiguousarray(C2 * W222.reshape(64, H), dtype=f)
    V02f = np.ascontiguousarray(
        (C2B * INV_S5) * (V022[:, :, 0] + V202[:, :, 0].T), dtype=f)      # [256,256]
    V222p = np.ascontiguousarray(C2B * V222[:, :, 0], dtype=f)            # [256,256]
    C222k = np.zeros((25, 8), dtype=f)                                    # [(i,j),k] padded
    C222k[:, :5] = C222.reshape(25, 5)
    ident = np.eye(128, dtype=f)
    return dict(s=np.ascontiguousarray(scalars, dtype=f), t=t,
                w000p=W000p, wpr=W_PR, w220p=W220p, w222p=W222p,
                v02f=V02f, v222p=V222p, c222k=C222k, ident=ident)


# C222 nonzeros per k: list of (i, j, coeff)
C222_NNZ = [[(i, j, float(C222[i, j, k]))
             for i in range(5) for j in range(5) if C222[i, j, k] != 0.0]
            for k in range(5)]


def build_nc(use_f32r=True, repeat=1, pool_products=False):
    import concourse.bacc as bacc
    import concourse.tile as tile
    import concourse.mybir as mybir

    f32 = mybir.dt.float32
    mmdt = mybir.dt.float32r if use_f32r else mybir.dt.float32
    MULT = mybir.AluOpType.mult
    ADD = mybir.AluOpType.add
    AX = mybir.AxisListType.X

    nc = bacc.Bacc("TRN2", target_bir_lowering=False, debug=False,
                   num_devices=N_CORES)

    def dram(name, shape, kind="ExternalInput", dt=None):
        return nc.dram_tensor(name, list(shape), dt or f32, kind=kind).ap()

    s_d = dram("s", [ZC, 128])
    t_d = dram("t", [ZC, 40])
    w000_d = dram("w000p", [NPACK, 256], dt=mmdt)
    wpr_d = dram("wpr", [1024, 256], dt=mmdt)
    w220_d = dram("w220p", [64, 256], dt=mmdt)
    w222_d = dram("w222p", [64, 256], dt=mmdt)
    v02f_d = dram("v02f", [256, 256], dt=mmdt)
    v222_d = dram("v222p", [256, 256], dt=mmdt)
    c222k_d = dram("c222k", [25, 8], dt=mmdt)
    ident_d = dram("ident", [128, 128], dt=mmdt)
    out_d = dram("out", [ZC, 5], kind="ExternalOutput")

    def mm(out, lhsT, rhs, start=True, stop=True):
        nc.tensor.matmul(out, lhsT, rhs, start=start, stop=stop)

    def stt(out, in0, in1, op1=MULT):
        nc.vector.scalar_tensor_tensor(out, in0, 1.0, in1, op0=MULT, op1=op1)

    def prod(out, in0, in1):
        if pool_products:
            nc.gpsimd.tensor_tensor(out, in0, in1, op=MULT)
        else:
            nc.vector.scalar_tensor_tensor(out, in0, 1.0, in1,
                                           op0=MULT, op1=MULT)

    from contextlib import ExitStack
    with tile.TileContext(nc) as tc, ExitStack() as es:
        es.enter_context(nc.allow_low_precision(
            reason="float32r tiles carry full fp32 bits; PSUM accumulation "
                   "is fp32"))
        if repeat > 1:
            es.enter_context(tc.For_i(0, repeat, 1))
        cpool = es.enter_context(tc.tile_pool(name="consts", bufs=1))
        work = es.enter_context(tc.tile_pool(name="work", bufs=1))
        wstream = es.enter_context(tc.tile_pool(name="wstream", bufs=5))
        s2tp = es.enter_context(tc.tile_pool(name="s2tp", bufs=4))
        scratch = es.enter_context(tc.tile_pool(name="scratch", bufs=2))
        sbig = es.enter_context(tc.tile_pool(name="sbig", bufs=2))
        ptr = es.enter_context(tc.tile_pool(name="ptr", bufs=3, space="PSUM"))
        py0 = es.enter_context(tc.tile_pool(name="py0", bufs=1, space="PSUM"))
        p256 = es.enter_context(tc.tile_pool(name="p256", bufs=3, space="PSUM"))
        apool = es.enter_context(tc.tile_pool(name="apool", bufs=3))

        dma = nc.sync.dma_start

        ident = cpool.tile([128, 128], mmdt, tag="ident")
        dma(out=ident, in_=ident_d)
        wpr_s = cpool.tile([128, 2048], mmdt, tag="wpr")
        for c in range(8):
            dma(out=wpr_s[:, c * 256:(c + 1) * 256],
                in_=wpr_d[c * 128:(c + 1) * 128, :])
        v02f_s = cpool.tile([128, 512], mmdt, tag="v02f")
        v222_s = cpool.tile([128, 512], mmdt, tag="v222")
        for h in range(2):
            dma(out=v02f_s[:, h * 256:(h + 1) * 256],
                in_=v02f_d[h * 128:(h + 1) * 128, :])
            dma(out=v222_s[:, h * 256:(h + 1) * 256],
                in_=v222_d[h * 128:(h + 1) * 128, :])
        w220_s = cpool.tile([64, 256], mmdt, tag="w220")
        dma(out=w220_s, in_=w220_d)
        w222_s = cpool.tile([64, 256], mmdt, tag="w222")
        dma(out=w222_s, in_=w222_d)
        c222k_s = cpool.tile([25, 8], mmdt, tag="c222k")
        dma(out=c222k_s, in_=c222k_d)

        s_s, t_s, s2p = [], [], []
        for tau in range(2):
            st = cpool.tile([128, 128], f32, tag=f"s{tau}")
            dma(out=st, in_=s_d[tau * 128:(tau + 1) * 128, :])
            s_s.append(st)
            tt = cpool.tile([128, 40], f32, tag=f"t{tau}")
            dma(out=tt, in_=t_d[tau * 128:(tau + 1) * 128, :])
            t_s.append(tt)
            s2 = work.tile([128, NPACK], mmdt, tag=f"s2p{tau}")
            s2p.append(s2)
            off = 0
            for gi, (u0, wv) in enumerate(GROUPS):
                in0 = st[:, u0:].unsqueeze(1).to_broadcast([128, GR, wv])
                in1 = st[:, u0:u0 + GR].unsqueeze(2).to_broadcast([128, GR, wv])
                out = s2[:, off:off + GR * wv].rearrange(
                    "p (a v) -> p a v", a=GR)
                prod(out, in0, in1)
                off += GR * wv

        # ---- small per-tile precomputations (T2, pair products) ----
        y0ps = [py0.tile([128, 256], f32, name=f"y0ps{tau}", tag=f"y0_{tau}")
                for tau in range(2)]
        t3_s, t2t_s, pp4_s, y2_s = [], [], [], []
        for tau in range(2):
            st, tt = s_s[tau], t_s[tau]
            t3 = tt.rearrange("p (l i) -> p l i", l=8)
            t3_s.append(t3)
            t2prod = scratch.tile([128, 320], f32, tag="t2prod")
            t2p3 = t2prod.rearrange("p (u v i) -> p (u v) i", u=8, v=8)
            for u in range(8):
                prod(t2p3[:, u * 8:(u + 1) * 8, :],
                     t3[:, u, :].unsqueeze(1).to_broadcast([128, 8, 5]),
                     t3)
            t2 = scratch.tile([128, 64], mmdt, tag="t2")
            nc.vector.tensor_reduce(
                t2, t2prod.rearrange("p (u v i) -> p (u v) i", u=8, v=8),
                axis=AX, op=ADD)
            pt2 = ptr.tile([64, 128], mmdt, tag="ptr")
            nc.tensor.transpose(pt2, t2, ident)
            t2t = scratch.tile([64, 128], mmdt, tag="t2t")
            nc.vector.tensor_copy(t2t, pt2)
            t2t_s.append(t2t)
            pairp = sbig.tile([128, 1600], f32, name=f"pairp_{tau}",
                              tag="pairp")
            prod(pairp.rearrange("p (a b) -> p a b", a=40),
                 tt.unsqueeze(2).to_broadcast([128, 40, 40]),
                 tt.unsqueeze(1).to_broadcast([128, 40, 40]))
            pp4_s.append(pairp.rearrange("p (u i v j) -> p u i v j",
                                         u=8, i=5, v=8))
            y2_s.append(work.tile([128, 1280], mmdt, name=f"y2_{tau}",
                                  tag=f"y2_{tau}"))

        def y2_slab(tau, k):
            """One y2 k-slab: A = s (x) t_k chunks -> transposed -> 8 W_PR
            matmuls + the W222 TP matmul into one PSUM slab."""
            st, t3, pp4, y2 = s_s[tau], t3_s[tau], pp4_s[tau], y2_s[tau]
            ak = apool.tile([128, 1024], mmdt, name="ak", tag="ak")
            stt(ak.rearrange("p (s l) -> p s l", s=128),
                st.unsqueeze(2).to_broadcast([128, 128, 8]),
                t3[:, :, k].unsqueeze(1).to_broadcast([128, 128, 8]))
            y2ps = p256.tile([128, 256], f32, name="y2ps", tag="p256")
            for half in range(2):
                ptq = ptr.tile([128, 512], mmdt, name="ptqa", tag="ptr")
                for j in range(4):
                    cc = half * 4 + j
                    nc.tensor.matmul(
                        ptq[:, j * 128:(j + 1) * 128],
                        ak[:, cc * 128:(cc + 1) * 128], ident,
                        is_transpose=True, start=(j == 0), stop=(j == 3))
                atq = s2tp.tile([128, 512], mmdt, name="atq", tag="s2t")
                nc.scalar.copy(out=atq, in_=ptq)
                for j in range(4):
                    cc = half * 4 + j
                    mm(y2ps, atq[:, j * 128:(j + 1) * 128],
                       wpr_s[:, cc * 256:(cc + 1) * 256],
                       start=(cc == 0), stop=False)
            tpk = scratch.tile([128, 64], mmdt, tag="tpk")
            for n, (i, j, coef) in enumerate(C222_NNZ[k]):
                sl = pp4[:, :, i, :, j]
                if n == 0:
                    nc.vector.tensor_scalar(tpk.rearrange(
                        "p (u v) -> p u v", u=8), sl, coef, None, op0=MULT)
                else:
                    nc.vector.scalar_tensor_tensor(
                        tpk.rearrange("p (u v) -> p u v", u=8), sl, coef,
                        tpk.rearrange("p (u v) -> p u v", u=8),
                        op0=MULT, op1=ADD)
            ptk = ptr.tile([64, 128], mmdt, tag="ptr")
            nc.tensor.transpose(ptk, tpk, ident)
            tpt = scratch.tile([64, 128], mmdt, tag="tpt")
            nc.vector.tensor_copy(tpt, ptk)
            mm(y2ps, tpt, w222_s, start=False, stop=True)
            nc.scalar.copy(out=y2[:, k * 256:(k + 1) * 256], in_=y2ps)

        # ---- V-path per tile, as units interleaved into the quad loop ----
        y2t_s = [sbig.tile([128, 1280], mmdt, name=f"y2t_{tau}", tag="y2t")
                 for tau in range(2)]
        d_s = [sbig.tile([128, 1280], f32, name=f"d_{tau}", tag="d")
               for tau in range(2)]
        fmat_s = [scratch.tile([128, 25], mmdt, name=f"fmat{tau}", tag="fmat")
                  for tau in range(2)]
        o2_s = [scratch.tile([128, 5], f32, name=f"o2_{tau}", tag="o2")
                for tau in range(2)]

        def u_y2t(tau):
            y2, y2t = y2_s[tau], y2t_s[tau]
            for h in range(2):
                ptq = ptr.tile([128, 512], mmdt, name="ptqy", tag="ptr")
                for i in range(4):
                    nc.tensor.matmul(
                        ptq[:, i * 128:(i + 1) * 128],
                        y2[:, i * 256 + h * 128:i * 256 + (h + 1) * 128],
                        ident, is_transpose=True, start=(i == 0),
                        stop=(i == 3))
                nc.scalar.copy(out=y2t[:, h * 640:h * 640 + 512], in_=ptq)
                pts = ptr.tile([128, 128], mmdt, name="pts", tag="ptr")
                nc.tensor.transpose(
                    pts, y2[:, 4 * 256 + h * 128:4 * 256 + (h + 1) * 128],
                    ident)
                nc.scalar.copy(out=y2t[:, h * 640 + 512:h * 640 + 640],
                               in_=pts)

        def u_d(tau):
            y2t, d = y2t_s[tau], d_s[tau]
            for i in range(5):
                dps = p256.tile([128, 256], f32, tag="p256")
                for h in range(2):
                    mm(dps, y2t[:, h * 640 + i * 128:h * 640 + (i + 1) * 128],
                       v222_s[:, h * 256:(h + 1) * 256],
                       start=(h == 0), stop=(h == 1))
                nc.scalar.copy(out=d[:, i * 256:(i + 1) * 256], in_=dps)

        def u_f(tau):
            y2, d, fmat = y2_s[tau], d_s[tau], fmat_s[tau]
            for i in range(5):
                fprod = sbig.tile([128, 1280], f32, name=f"fprod{i}",
                                  tag="fprod")
                stt(fprod.rearrange("p (j v) -> p j v", j=5),
                    d[:, i * 256:(i + 1) * 256].unsqueeze(1)
                     .to_broadcast([128, 5, 256]),
                    y2.rearrange("p (j v) -> p j v", j=5))
                nc.vector.tensor_reduce(
                    fmat[:, i * 5:(i + 1) * 5],
                    fprod.rearrange("p (j v) -> p j v", j=5), axis=AX, op=ADD)
            ptf = ptr.tile([25, 128], mmdt, tag="ptr")
            nc.tensor.transpose(ptf, fmat, ident)
            ft = scratch.tile([25, 128], mmdt, tag="ft")
            nc.vector.tensor_copy(ft, ptf)
            o2ps = p256.tile([128, 8], f32, tag="p256")
            mm(o2ps, ft, c222k_s)
            nc.vector.tensor_copy(o2_s[tau], o2ps[:, :5])

        vunits = [lambda: u_y2t(0), lambda: u_y2t(1),
                  lambda: u_d(0), lambda: u_d(1),
                  lambda: u_f(0), lambda: u_f(1)]

        # ---- main loop: W000 quads with y2 slabs software-pipelined ----
        for q in range(NCHUNK // 4):
            wq = wstream.tile([128, 1024], mmdt, name="wq", tag="w000c")
            dma(out=wq.rearrange("p (c w) -> p c w", c=4),
                in_=w000_d[q * 512:(q + 1) * 512, :]
                .rearrange("(c p) w -> p c w", p=128))
            wchs = [wq[:, j * 256:(j + 1) * 256] for j in range(4)]
            for tau in range(2):
                ptq = ptr.tile([128, 512], mmdt, name="ptq", tag="ptr")
                for j in range(4):
                    c = 4 * q + j
                    nc.tensor.matmul(
                        ptq[:, j * 128:(j + 1) * 128],
                        s2p[tau][:, c * 128:(c + 1) * 128], ident,
                        is_transpose=True, start=(j == 0), stop=(j == 3))
                s2tq = s2tp.tile([128, 512], mmdt, name="s2tq", tag="s2t")
                nc.scalar.copy(out=s2tq, in_=ptq)
                for j in range(4):
                    mm(y0ps[tau], s2tq[:, j * 128:(j + 1) * 128], wchs[j],
                       start=(q == 0 and j == 0), stop=False)
            if q < 10:
                y2_slab(q % 2, q // 2)
            elif q - 11 >= 0 and q - 11 < len(vunits):
                vunits[q - 11]()

        for u in vunits[(NCHUNK // 4) - 11:]:
            u()

        # ---- post-y0 tail ----
        for tau in range(2):
            mm(y0ps[tau], t2t_s[tau], w220_s, start=False, stop=True)
            y0 = work.tile([128, 256], mmdt, name=f"y0_{tau}", tag="y0")
            nc.vector.tensor_copy(y0, y0ps[tau])
            y0t = work.tile([128, 256], mmdt, name=f"y0t_{tau}", tag="y0t")
            pth0 = ptr.tile([128, 256], mmdt, name="pth0", tag="ptr")
            for h in range(2):
                nc.tensor.matmul(pth0[:, h * 128:(h + 1) * 128],
                                 y0[:, h * 128:(h + 1) * 128], ident,
                                 is_transpose=True, start=(h == 0),
                                 stop=(h == 1))
            nc.vector.tensor_copy(y0t, pth0)
            afps = p256.tile([128, 256], f32, tag="p256")
            for h in range(2):
                mm(afps, y0t[:, h * 128:(h + 1) * 128],
                   v02f_s[:, h * 256:(h + 1) * 256],
                   start=(h == 0), stop=(h == 1))
            af = scratch.tile([128, 256], f32, tag="af")
            nc.vector.tensor_copy(af, afps)
            y2 = y2_s[tau]
            oprod = sbig.tile([128, 1280], f32, name="oprod", tag="oprod")
            prod(oprod.rearrange("p (k v) -> p k v", k=5),
                 y2.rearrange("p (k v) -> p k v", k=5),
                 af.unsqueeze(1).to_broadcast([128, 5, 256]))
            o01 = scratch.tile([128, 5], f32, tag="o01")
            nc.vector.tensor_reduce(
                o01, oprod.rearrange("p (k v) -> p k v", k=5), axis=AX, op=ADD)
            outt = scratch.tile([128, 5], f32, tag="outt")
            nc.vector.scalar_tensor_tensor(outt, o2_s[tau], 1.0, o01,
                                           op0=MULT, op1=ADD)
            dma(out=out_d[tau * 128:(tau + 1) * 128, :], in_=outt)

    nc.compile()
    return nc


_CACHE = {}


def _get_nc(use_f32r=True):
    key = ("nc", use_f32r)
    if key not in _CACHE:
        _CACHE[key] = build_nc(use_f32r)
    return _CACHE[key]


def kernel(**inputs):
    from concourse.bass_utils import run_bass_kernel_spmd

    prep = host_prep(**{k: np.asarray(v) for k, v in inputs.items()})
    nc = _get_nc()
    shared = {k: prep[k] for k in ("w000p", "wpr", "w220p", "w222p",
                                   "v02f", "v222p", "c222k", "ident")}
    in_maps = []
    for c in range(N_CORES):
        rows = slice(c * ZC, (c + 1) * ZC)
        m = dict(shared)
        m["s"] = np.ascontiguousarray(prep["s"][rows])
        m["t"] = np.ascontiguousarray(prep["t"][rows])
        in_maps.append(m)
    res = run_bass_kernel_spmd(nc, in_maps, list(range(N_CORES)))
    out = np.concatenate([res.results[c]["out"] for c in range(N_CORES)], 0)
    return out.astype(np.float32)

